# revision 1
# baseline (speedup 1.0000x reference)
"""PairEmbedding Bass kernel for 8 TRN2 NeuronCores.

out[b,i,j,:] = Co[b,j,:] + Cp[b,i,:] + sep(b,i,j) * w_sep
  Co[j] = se_j @ W1 + [0 | pe_j]
  Cp[i] = se_i @ W2 + b_proj + [pe_i | 0]
  sep(i,j) = ln(|aa_i - aa_j| + 1)
where se = emb_table[seq], pe = pos_table[aa_idx], W1 = W_proj[0:144],
W2 = W_proj[144:288], w_sep = W_proj[288].

Sharding: core c -> batch b = c//2, row block i in [128*(c%2), 128*(c%2)+128),
all 256 j. Per-core output (128, 256, 288) f32.
"""

import math
from contextlib import ExitStack

import numpy as np

from concourse import bacc, bass, mybir, tile
from concourse.bass_utils import run_bass_kernel_spmd

dt = mybir.dt
AF = mybir.ActivationFunctionType
ALU = mybir.AluOpType

B = 4
L = 256
D_PAIR = 288
D_HALF = 144
MAX_LEN = 260
VOCAB = 21
IH = 128          # i rows per core
JG = 8            # j's per output DMA group
N_CORES = 8


def _pos_enc_table() -> np.ndarray:
    idx = np.arange(0, D_HALF, 2, dtype=np.float32)
    t = (np.float32(math.log(10000.0)) * idx) / np.float32(D_HALF)
    denom = np.exp(t, dtype=np.float32)
    pos = np.arange(MAX_LEN, dtype=np.float32)[:, None]
    pe = np.zeros((MAX_LEN, D_HALF), dtype=np.float32)
    pe[:, 0::2] = np.sin(pos / denom, dtype=np.float32)
    pe[:, 1::2] = np.cos(pos / denom, dtype=np.float32)
    return pe


def _bcast(ap_src, nparts: int):
    return bass.AP(
        tensor=ap_src.tensor, offset=ap_src.offset, ap=[[0, nparts], *ap_src.ap]
    )


def build(stage: str = "full", repeat: int = 1, variant: str = "") -> bass.Bass:
    nc = bacc.Bacc("TRN2", target_bir_lowering=False)

    seqb_d = nc.dram_tensor("seqb", [L], dt.int32, kind="ExternalInput")
    seqi_d = nc.dram_tensor("seqi", [IH], dt.int32, kind="ExternalInput")
    aab_d = nc.dram_tensor("aab", [L], dt.int32, kind="ExternalInput")
    aai_d = nc.dram_tensor("aai", [IH], dt.int32, kind="ExternalInput")
    emb_d = nc.dram_tensor("emb", [VOCAB, D_HALF], dt.float32, kind="ExternalInput")
    wp_d = nc.dram_tensor("wp", [D_PAIR + 1, D_PAIR], dt.float32, kind="ExternalInput")
    bp_d = nc.dram_tensor("bp", [D_PAIR], dt.float32, kind="ExternalInput")
    out_d = nc.dram_tensor("out", [IH, L, D_PAIR], dt.float32, kind="ExternalOutput")

    # pos-table gather sources, pre-arranged on host: chunk c of <=128 pos
    # rows on partitions, channel slice [0:144] (posL, pe_i) or [144:288]
    # (posR, pe_j), zero elsewhere.
    pos_np = _pos_enc_table()
    posL_np = np.zeros((128, 3 * D_PAIR), dtype=np.float32)
    posR_np = np.zeros((128, 3 * D_PAIR), dtype=np.float32)
    for c in range(3):
        rows = 128 if c < 2 else MAX_LEN - 256
        chunk = pos_np[c * 128 : c * 128 + rows, :]
        posL_np[0:rows, c * D_PAIR : c * D_PAIR + D_HALF] = chunk
        posR_np[0:rows, c * D_PAIR + D_HALF : (c + 1) * D_PAIR] = chunk
    posL_d = nc.inline_tensor(posL_np, "posL_c")
    posR_d = nc.inline_tensor(posR_np, "posR_c")
    iota_np = (
        np.arange(128, dtype=np.float32)[:, None]
        + 128.0 * np.arange(3, dtype=np.float32)[None, :]
    ).astype(np.float32)
    iota_d = nc.inline_tensor(iota_np, "iota")

    with tile.TileContext(nc) as tc, ExitStack() as ctx:
        persist = ctx.enter_context(tc.tile_pool(name="persist", bufs=1))

        # persistent tiles consumed by the j-loop
        flat_t = persist.tile([2, L * D_PAIR], dt.bfloat16, tag="flat")
        ones_t = persist.tile([2, IH], dt.bfloat16, tag="ones")
        cp_t = persist.tile([IH, D_PAIR], dt.float32, tag="cpt")
        wsep_t = persist.tile([IH, D_PAIR], dt.float32, tag="wsep")
        sep_t = persist.tile([IH, L], dt.float32, tag="sept")

        nc.vector.memset(ones_t, 1.0)

        with ExitStack() as pre:
            scr = pre.enter_context(tc.tile_pool(name="scr", bufs=1))
            psc = pre.enter_context(tc.tile_pool(name="psc", bufs=1, space="PSUM"))

            # ---- input loads ----
            iota_t = scr.tile([128, 3], dt.float32, tag="iota")
            nc.sync.dma_start(iota_t, iota_d[:, :])

            emb_t = scr.tile([VOCAB, D_HALF], dt.float32, tag="emb")
            nc.sync.dma_start(emb_t, emb_d[:, :])

            w1a = scr.tile([128, D_PAIR], dt.float32, tag="w1a")
            nc.sync.dma_start(w1a, wp_d[0:128, :])
            w1b = scr.tile([16, D_PAIR], dt.float32, tag="w1b")
            nc.sync.dma_start(w1b, wp_d[128:144, :])
            w2a = scr.tile([128, D_PAIR], dt.float32, tag="w2a")
            nc.sync.dma_start(w2a, wp_d[144:272, :])
            w2b = scr.tile([16, D_PAIR], dt.float32, tag="w2b")
            nc.sync.dma_start(w2b, wp_d[272:288, :])
            nc.sync.dma_start(wsep_t, _bcast(wp_d[288:289, :], 128))

            bp_t = scr.tile([1, D_PAIR], dt.float32, tag="bp")
            nc.sync.dma_start(bp_t, bp_d[:])

            aaB_i = scr.tile([128, L], dt.int32, tag="aaBi")
            nc.sync.dma_start(aaB_i, _bcast(aab_d[:], 128))
            seqB_i = scr.tile([VOCAB, L], dt.int32, tag="seqBi")
            nc.sync.dma_start(seqB_i, _bcast(seqb_d[:], VOCAB))
            seqI_i = scr.tile([VOCAB, IH], dt.int32, tag="seqIi")
            nc.sync.dma_start(seqI_i, _bcast(seqi_d[:], VOCAB))
            aaIB_i = scr.tile([128, IH], dt.int32, tag="aaIBi")
            nc.sync.dma_start(aaIB_i, _bcast(aai_d[:], 128))
            aaCol_i = scr.tile([IH, 1], dt.int32, tag="aaColi")
            nc.sync.dma_start(aaCol_i, aai_d[:])

            posL = scr.tile([128, 3 * D_PAIR], dt.float32, tag="posL")
            nc.sync.dma_start(posL, posL_d[:, :])
            posR = scr.tile([128, 3 * D_PAIR], dt.float32, tag="posR")
            nc.sync.dma_start(posR, posR_d[:, :])

            # ---- int -> f32 casts ----
            aaB_f = scr.tile([128, L], dt.float32, tag="aaBf")
            nc.vector.tensor_copy(aaB_f, aaB_i)
            seqB_f = scr.tile([VOCAB, L], dt.float32, tag="seqBf")
            nc.vector.tensor_copy(seqB_f, seqB_i)
            seqI_f = scr.tile([VOCAB, IH], dt.float32, tag="seqIf")
            nc.vector.tensor_copy(seqI_f, seqI_i)
            aaIB_f = scr.tile([128, IH], dt.float32, tag="aaIBf")
            nc.vector.tensor_copy(aaIB_f, aaIB_i)
            aaCol_f = scr.tile([IH, 1], dt.float32, tag="aaColf")
            nc.vector.tensor_copy(aaCol_f, aaCol_i)

            # ---- one-hots ----
            ohSeq = scr.tile([VOCAB, L], dt.float32, tag="ohSeq")
            nc.vector.tensor_scalar(
                ohSeq, seqB_f, iota_t[0:VOCAB, 0:1], None, ALU.is_equal
            )
            ohSeqI = scr.tile([VOCAB, IH], dt.float32, tag="ohSeqI")
            nc.vector.tensor_scalar(
                ohSeqI, seqI_f, iota_t[0:VOCAB, 0:1], None, ALU.is_equal
            )
            ohP = []
            ohPi = []
            for c in range(3):
                t = scr.tile([128, L], dt.float32, tag=f"ohP{c}", name=f"ohP{c}")
                nc.vector.tensor_scalar(t, aaB_f, iota_t[:, c : c + 1], None, ALU.is_equal)
                ohP.append(t)
                ti = scr.tile([128, IH], dt.float32, tag=f"ohPi{c}", name=f"ohPi{c}")
                nc.vector.tensor_scalar(
                    ti, aaIB_f, iota_t[:, c : c + 1], None, ALU.is_equal
                )
                ohPi.append(ti)

            # ---- seT = emb^T gathered by seq: (144, L) split 128+16 rows ----
            seT_a_ps = psc.tile([128, L], dt.float32, tag="seTaP")
            nc.tensor.matmul(seT_a_ps, emb_t[:, 0:128], ohSeq, start=True, stop=True)
            seT_b_ps = psc.tile([16, L], dt.float32, tag="seTbP")
            nc.tensor.matmul(
                seT_b_ps, emb_t[:, 128:D_HALF], ohSeq, start=True, stop=True
            )
            seT_a = scr.tile([128, L], dt.float32, tag="seTa")
            nc.vector.tensor_copy(seT_a, seT_a_ps)
            seT_b = scr.tile([16, L], dt.float32, tag="seTb")
            nc.vector.tensor_copy(seT_b, seT_b_ps)

            seTi_a_ps = psc.tile([128, IH], dt.float32, tag="seTiaP")
            nc.tensor.matmul(
                seTi_a_ps, emb_t[:, 0:128], ohSeqI, start=True, stop=True
            )
            seTi_b_ps = psc.tile([16, IH], dt.float32, tag="seTibP")
            nc.tensor.matmul(
                seTi_b_ps, emb_t[:, 128:D_HALF], ohSeqI, start=True, stop=True
            )
            seTi_a = scr.tile([128, IH], dt.float32, tag="seTia")
            nc.vector.tensor_copy(seTi_a, seTi_a_ps)
            seTi_b = scr.tile([16, IH], dt.float32, tag="seTib")
            nc.vector.tensor_copy(seTi_b, seTi_b_ps)

            # ---- Co halves -> bf16 hi/lo -> flat layout on partitions 0/32 ----
            for h in range(2):
                co_ps = psc.tile(
                    [128, D_PAIR], dt.float32, tag=f"co{h}", name=f"co{h}"
                )
                sl = slice(h * 128, (h + 1) * 128)
                nc.tensor.matmul(co_ps, seT_a[:, sl], w1a, start=True, stop=False)
                nc.tensor.matmul(co_ps, seT_b[:, sl], w1b, start=False, stop=False)
                for c in range(3):
                    nc.tensor.matmul(
                        co_ps,
                        ohP[c][:, sl],
                        posR[:, c * D_PAIR : (c + 1) * D_PAIR],
                        start=False,
                        stop=(c == 2),
                    )
                co_hi = scr.tile(
                    [128, D_PAIR], dt.bfloat16, tag=f"cohi{h}", name=f"cohi{h}"
                )
                nc.vector.tensor_copy(co_hi, co_ps)
                co_lo = scr.tile(
                    [128, D_PAIR], dt.bfloat16, tag=f"colo{h}", name=f"colo{h}"
                )
                nc.vector.tensor_sub(co_lo, co_ps, co_hi)
                dst = slice(h * 128 * D_PAIR, (h * 128 + 128) * D_PAIR)
                nc.sync.dma_start(flat_t[0:1, dst], co_hi)
                nc.sync.dma_start(flat_t[1:2, dst], co_lo)

            # ---- Cp = se_i @ W2 + b_proj + [pe_i | 0] ----
            ones_f = scr.tile([1, IH], dt.float32, tag="onesf")
            nc.vector.memset(ones_f, 1.0)
            cp_ps = psc.tile([128, D_PAIR], dt.float32, tag="cpP")
            nc.tensor.matmul(cp_ps, seTi_a, w2a, start=True, stop=False)
            nc.tensor.matmul(cp_ps, seTi_b, w2b, start=False, stop=False)
            for c in range(3):
                nc.tensor.matmul(
                    cp_ps,
                    ohPi[c],
                    posL[:, c * D_PAIR : (c + 1) * D_PAIR],
                    start=False,
                    stop=False,
                )
            nc.tensor.matmul(cp_ps, ones_f, bp_t, start=False, stop=True)
            nc.vector.tensor_copy(cp_t, cp_ps)

            # ---- sep = ln(|aa_j - aa_i| + 1) ----
            dist_t = scr.tile([IH, L], dt.float32, tag="dist")
            nc.vector.tensor_scalar(dist_t, aaB_f, aaCol_f, None, ALU.subtract)
            abs_t = scr.tile([IH, L], dt.float32, tag="abs")
            nc.scalar.activation(abs_t, dist_t, AF.Abs)
            nc.scalar.activation(sep_t, abs_t, AF.Ln, bias=1.0)

        if stage == "setup":
            # dump a few persistent tiles into out rows and stop
            dbg = ctx.enter_context(tc.tile_pool(name="dbg", bufs=1))
            dbf = dbg.tile([IH, D_PAIR], dt.float32, tag="dbf")
            nc.vector.tensor_copy(dbf, cp_t)
            nc.sync.dma_start(out_d[:, 0:1, :], dbf)
            nc.vector.tensor_copy(dbf, wsep_t)
            nc.sync.dma_start(out_d[:, 1:2, :], dbf)
            return nc

        # ---- j loop ----
        psj = ctx.enter_context(tc.tile_pool(name="psj", bufs=8, space="PSUM"))
        obp = ctx.enter_context(tc.tile_pool(name="obp", bufs=2))
        ngroups = int(stage[5:]) if stage.startswith("jloop") else L // JG
        if variant == "dmaonly":
            obs = []
            for k in range(2):
                t = obp.tile([IH, JG * D_PAIR], dt.float32, tag="ob", name="ob")
                nc.vector.memset(t, 0.5)
                obs.append(t)
            for g in range(ngroups * repeat):
                g = g % ngroups
                eng = nc.sync if g % 2 == 0 else nc.scalar
                eng.dma_start(out_d[:, g * JG : (g + 1) * JG, :], obs[g % 2])
            return nc
        for g in range(ngroups * repeat):
            g = g % ngroups
            ob = obp.tile([IH, JG * D_PAIR], dt.float32, tag="ob", name="ob")
            for jj in range(JG):
                j = g * JG + jj
                ps = psj.tile([IH, D_PAIR], dt.float32, tag="ps", name="ps")
                nc.tensor.matmul(
                    ps,
                    ones_t[0:2, :],
                    flat_t[0:2, j * D_PAIR : (j + 1) * D_PAIR],
                    start=True,
                    stop=True,
                )
                osl = ob[:, jj * D_PAIR : (jj + 1) * D_PAIR]
                if variant == "nostt":
                    nc.vector.tensor_copy(osl, ps)
                elif variant == "sttsb":
                    nc.vector.scalar_tensor_tensor(
                        osl, wsep_t, sep_t[:, j : j + 1], cp_t, ALU.mult, ALU.add
                    )
                else:
                    nc.vector.scalar_tensor_tensor(
                        osl, wsep_t, sep_t[:, j : j + 1], ps, ALU.mult, ALU.add
                    )
                if variant not in ("nopool", "nostt", "sttsb"):
                    nc.gpsimd.tensor_add(osl, osl, cp_t)
            if variant != "nodma":
                eng = nc.sync if g % 2 == 0 else nc.scalar
                eng.dma_start(out_d[:, g * JG : (g + 1) * JG, :], ob)

    return nc


_NC_CACHE = []


def make_in_maps(seq, aa_idx, emb_table, W_proj, b_proj):
    seq = np.asarray(seq, dtype=np.int32)
    aa_idx = np.asarray(aa_idx, dtype=np.int32)
    emb_table = np.ascontiguousarray(np.asarray(emb_table, dtype=np.float32))
    W_proj = np.ascontiguousarray(np.asarray(W_proj, dtype=np.float32))
    b_proj = np.ascontiguousarray(np.asarray(b_proj, dtype=np.float32))
    in_maps = []
    for c in range(N_CORES):
        b, ih = c // 2, c % 2
        in_maps.append(
            {
                "seqb": np.ascontiguousarray(seq[b]),
                "seqi": np.ascontiguousarray(seq[b, ih * IH : (ih + 1) * IH]),
                "aab": np.ascontiguousarray(aa_idx[b]),
                "aai": np.ascontiguousarray(aa_idx[b, ih * IH : (ih + 1) * IH]),
                "emb": emb_table,
                "wp": W_proj,
                "bp": b_proj,
            }
        )
    return in_maps


def gather_out(results) -> np.ndarray:
    out = np.empty((B, L, L, D_PAIR), dtype=np.float32)
    for c in range(N_CORES):
        b, ih = c // 2, c % 2
        out[b, ih * IH : (ih + 1) * IH] = np.asarray(results[c]["out"])
    return out


def kernel(seq, aa_idx, emb_table, W_proj, b_proj) -> np.ndarray:
    if not _NC_CACHE:
        nc = build()
        nc.finalize()
        _NC_CACHE.append(nc)
    nc = _NC_CACHE[0]
    in_maps = make_in_maps(seq, aa_idx, emb_table, W_proj, b_proj)
    res = run_bass_kernel_spmd(nc, in_maps, core_ids=list(range(N_CORES)))
    return gather_out(res.results)



# revision 8
# speedup vs baseline: 1.7990x; 1.7990x over previous
"""PairEmbedding Bass kernel for 8 TRN2 NeuronCores.

out[b,i,j,:] = Co[b,j,:] + Cp[b,i,:] + sep(b,i,j) * w_sep
  Co[j] = se_j @ W1 + [0 | pe_j]
  Cp[i] = se_i @ W2 + b_proj + [pe_i | 0]
  sep(i,j) = ln(|aa_i - aa_j| + 1)
where se = emb_table[seq], pe = pos_table[aa_idx], W1 = W_proj[0:144],
W2 = W_proj[144:288], w_sep = W_proj[288].

Sharding: core c -> batch b = c//2, row block i in [128*(c%2), 128*(c%2)+128),
all 256 j. Per-core output (128, 256, 288) f16 (upcast to f32 on host).

Steady state per 4-j group g:
  PE: 4 matmuls ps_j = Gall[:, g].T @ R_ALL[:, j*288:+288] where the K=6
      contraction rows are [1, 1, sep_g0..sep_g3] against
      [co_hi, co_lo, wsep-pattern rows], giving Co[j] + sep[:,j]*wsep in
      f32 PSUM at bank-aligned slots.
  ACT/GPS/DVE (scheduled): convert PSUM->f16 and add cp_rep -> ob f16.
  SP: DMA ob -> out[:, 4g:4g+4, :].
"""

import math
from contextlib import ExitStack

import numpy as np

from concourse import bacc, bass, mybir, tile
from concourse.bass_utils import run_bass_kernel_spmd

dt = mybir.dt
AF = mybir.ActivationFunctionType
ALU = mybir.AluOpType

B = 4
L = 256
D_PAIR = 288
D_HALF = 144
MAX_LEN = 260
VOCAB = 21
IH = 128          # i rows per core
JG = 4            # j's per group (PSUM: one bank slot per j, 2 groups in flight)
NG = L // JG      # 64 groups
N_CORES = 8

# per-group engine schedule (GPSIMD cannot touch PSUM):
#   A = ACT converts PSUM->f16, DVE adds cp_rep
#   H = ACT converts PSUM->f16, GPSIMD adds cp_rep
#   D = DVE fused convert+add straight from PSUM
SCHED16 = "DAHDAHDAHDAHDAHD"


def _pos_enc_table() -> np.ndarray:
    idx = np.arange(0, D_HALF, 2, dtype=np.float32)
    t = (np.float32(math.log(10000.0)) * idx) / np.float32(D_HALF)
    denom = np.exp(t, dtype=np.float32)
    pos = np.arange(MAX_LEN, dtype=np.float32)[:, None]
    pe = np.zeros((MAX_LEN, D_HALF), dtype=np.float32)
    pe[:, 0::2] = np.sin(pos / denom, dtype=np.float32)
    pe[:, 1::2] = np.cos(pos / denom, dtype=np.float32)
    return pe


def _bcast(ap_src, nparts: int):
    return bass.AP(
        tensor=ap_src.tensor, offset=ap_src.offset, ap=[[0, nparts], *ap_src.ap]
    )


def _pstride(ap_src, step: int, count: int):
    """Prepend a strided partition dim [step, count] to a DRAM AP."""
    return bass.AP(
        tensor=ap_src.tensor, offset=ap_src.offset, ap=[[step, count], *ap_src.ap]
    )


def _fstride(ap_src, outer_step: int, outer_count: int, inner_count: int):
    """2-level free AP [[outer_step, outer_count], [1, inner_count]] on a
    [P, n]-contiguous slice."""
    return bass.AP(
        tensor=ap_src.tensor,
        offset=ap_src.offset,
        ap=[ap_src.ap[0], [outer_step, outer_count], [1, inner_count]],
    )


def build(stage: str = "full", repeat: int = 1, variant: str = "") -> bass.Bass:
    nc = bacc.Bacc("TRN2", target_bir_lowering=False)

    seqb_d = nc.dram_tensor("seqb", [L], dt.int32, kind="ExternalInput")
    seqi_d = nc.dram_tensor("seqi", [IH], dt.int32, kind="ExternalInput")
    aab_d = nc.dram_tensor("aab", [L], dt.int32, kind="ExternalInput")
    aai_d = nc.dram_tensor("aai", [IH], dt.int32, kind="ExternalInput")
    emb_d = nc.dram_tensor("emb", [VOCAB, D_HALF], dt.float32, kind="ExternalInput")
    wp_d = nc.dram_tensor("wp", [D_PAIR + 1, D_PAIR], dt.float32, kind="ExternalInput")
    bp_d = nc.dram_tensor("bp", [D_PAIR], dt.float32, kind="ExternalInput")
    out_d = nc.dram_tensor("out", [IH, L, D_PAIR], dt.float16, kind="ExternalOutput")

    # pos-table gather sources, pre-arranged on host: chunk c of <=128 pos
    # rows on partitions, channel slice [0:144] (posL, pe_i) or [144:288]
    # (posR, pe_j), zero elsewhere.
    pos_np = _pos_enc_table()
    posL_np = np.zeros((128, 3 * D_PAIR), dtype=np.float32)
    posR_np = np.zeros((128, 3 * D_PAIR), dtype=np.float32)
    for c in range(3):
        rows = 128 if c < 2 else MAX_LEN - 256
        chunk = pos_np[c * 128 : c * 128 + rows, :]
        posL_np[0:rows, c * D_PAIR : c * D_PAIR + D_HALF] = chunk
        posR_np[0:rows, c * D_PAIR + D_HALF : (c + 1) * D_PAIR] = chunk
    posL_d = nc.inline_tensor(posL_np, "posL_c")
    posR_d = nc.inline_tensor(posR_np, "posR_c")
    iota_np = (
        np.arange(128, dtype=np.float32)[:, None]
        + 128.0 * np.arange(3, dtype=np.float32)[None, :]
    ).astype(np.float32)
    iota_d = nc.inline_tensor(iota_np, "iota")
    ones2_d = nc.inline_tensor(np.ones((2, NG * IH), dtype=np.float16), "ones2")

    sched = (SCHED16 * 4)[:NG]
    for v in variant.split("+"):
        if v.startswith("sched"):
            pat = v[5:]
            sched = (pat * ((NG // len(pat)) + 1))[:NG]

    with tile.TileContext(nc) as tc, ExitStack() as ctx:
        persist = ctx.enter_context(tc.tile_pool(name="persist", bufs=1))
        psp = ctx.enter_context(tc.tile_pool(name="psp", bufs=1, space="PSUM"))

        # persistent tiles consumed by the j-loop
        # R_ALL rows: 0=co_hi, 1=co_lo, 2..5 = wsep at j%4==jj (f16)
        rall_t = persist.tile([6, L * D_PAIR], dt.float16, tag="rall")
        # Gall rows: 0,1=ones, 2..5 = sep[i, 4g+jj]; free = g*128 + i
        gall_t = persist.tile([6, NG * IH], dt.float16, tag="gall")
        cp_rep = persist.tile([IH, JG * D_PAIR], dt.float16, tag="cprep")
        ps_all = psp.tile([128, 4096], dt.float32, tag="psall")

        with ExitStack() as pre:
            scr = pre.enter_context(tc.tile_pool(name="scr", bufs=1))

            # ---- input loads ----
            iota_t = scr.tile([128, 3], dt.float32, tag="iota")
            nc.sync.dma_start(iota_t, iota_d[:, :])

            emb_t = scr.tile([VOCAB, D_HALF], dt.float32, tag="emb")
            nc.sync.dma_start(emb_t, emb_d[:, :])

            w1a = scr.tile([128, D_PAIR], dt.float32, tag="w1a")
            nc.sync.dma_start(w1a, wp_d[0:128, :])
            w1b = scr.tile([16, D_PAIR], dt.float32, tag="w1b")
            nc.sync.dma_start(w1b, wp_d[128:144, :])
            w2a = scr.tile([128, D_PAIR], dt.float32, tag="w2a")
            nc.sync.dma_start(w2a, wp_d[144:272, :])
            w2b = scr.tile([16, D_PAIR], dt.float32, tag="w2b")
            nc.sync.dma_start(w2b, wp_d[272:288, :])
            wsep_f = scr.tile([1, D_PAIR], dt.float32, tag="wsepf")
            nc.sync.dma_start(wsep_f, wp_d[288:289, :])

            bp_t = scr.tile([1, D_PAIR], dt.float32, tag="bp")
            nc.sync.dma_start(bp_t, bp_d[:])

            seqB_i = scr.tile([VOCAB, L], dt.int32, tag="seqBi")
            nc.sync.dma_start(seqB_i, _bcast(seqb_d[:], VOCAB))
            seqI_i = scr.tile([VOCAB, IH], dt.int32, tag="seqIi")
            nc.sync.dma_start(seqI_i, _bcast(seqi_d[:], VOCAB))
            aaIB_i = scr.tile([128, IH], dt.int32, tag="aaIBi")
            nc.sync.dma_start(aaIB_i, _bcast(aai_d[:], 128))
            # aa_j columns for j = 4g+jj, g on partitions
            aaB4_i = scr.tile([64, JG], dt.int32, tag="aaB4i")
            for jj in range(JG):
                nc.sync.dma_start(
                    aaB4_i[:, jj : jj + 1], _pstride(aab_d[jj : jj + 1], JG, 64)
                )

            posL = scr.tile([128, 3 * D_PAIR], dt.float32, tag="posL")
            nc.sync.dma_start(posL, posL_d[:, :])
            posR = scr.tile([128, 3 * D_PAIR], dt.float32, tag="posR")
            nc.sync.dma_start(posR, posR_d[:, :])

            nc.sync.dma_start(gall_t[0:2, :], ones2_d[:, :])

            # ---- int -> f32 casts ----
            seqB_f = scr.tile([VOCAB, L], dt.float32, tag="seqBf")
            nc.vector.tensor_copy(seqB_f, seqB_i)
            seqI_f = scr.tile([VOCAB, IH], dt.float32, tag="seqIf")
            nc.vector.tensor_copy(seqI_f, seqI_i)
            aaIB_f = scr.tile([128, IH], dt.float32, tag="aaIBf")
            nc.vector.tensor_copy(aaIB_f, aaIB_i)
            aaB4_f = scr.tile([64, JG], dt.float32, tag="aaB4f")
            nc.vector.tensor_copy(aaB4_f, aaB4_i)

            # ---- one-hots ----
            ohSeq = scr.tile([VOCAB, L], dt.float32, tag="ohSeq")
            nc.vector.tensor_scalar(
                ohSeq, seqB_f, iota_t[0:VOCAB, 0:1], None, ALU.is_equal
            )
            ohSeqI = scr.tile([VOCAB, IH], dt.float32, tag="ohSeqI")
            nc.vector.tensor_scalar(
                ohSeqI, seqI_f, iota_t[0:VOCAB, 0:1], None, ALU.is_equal
            )
            aaB_i = scr.tile([128, L], dt.int32, tag="aaBi")
            nc.sync.dma_start(aaB_i, _bcast(aab_d[:], 128))
            aaB_f = scr.tile([128, L], dt.float32, tag="aaBf")
            nc.vector.tensor_copy(aaB_f, aaB_i)
            ohP = []
            ohPi = []
            for c in range(3):
                t = scr.tile([128, L], dt.float32, tag=f"ohP{c}", name=f"ohP{c}")
                nc.vector.tensor_scalar(t, aaB_f, iota_t[:, c : c + 1], None, ALU.is_equal)
                ohP.append(t)
                ti = scr.tile([128, IH], dt.float32, tag=f"ohPi{c}", name=f"ohPi{c}")
                nc.vector.tensor_scalar(
                    ti, aaIB_f, iota_t[:, c : c + 1], None, ALU.is_equal
                )
                ohPi.append(ti)

            # ---- seT = emb^T gathered by seq: (144, L) split 128+16 rows ----
            seT_a_ps = ps_all[:, 0:L]
            nc.tensor.matmul(seT_a_ps, emb_t[:, 0:128], ohSeq, start=True, stop=True)
            seT_b_ps = ps_all[0:16, 256 : 256 + L]
            nc.tensor.matmul(
                seT_b_ps, emb_t[:, 128:D_HALF], ohSeq, start=True, stop=True
            )
            seT_a = scr.tile([128, L], dt.float32, tag="seTa")
            nc.vector.tensor_copy(seT_a, seT_a_ps)
            seT_b = scr.tile([16, L], dt.float32, tag="seTb")
            nc.vector.tensor_copy(seT_b, seT_b_ps)

            seTi_a_ps = ps_all[:, 512 : 512 + IH]
            nc.tensor.matmul(
                seTi_a_ps, emb_t[:, 0:128], ohSeqI, start=True, stop=True
            )
            seTi_b_ps = ps_all[0:16, 640 : 640 + IH]
            nc.tensor.matmul(
                seTi_b_ps, emb_t[:, 128:D_HALF], ohSeqI, start=True, stop=True
            )
            seTi_a = scr.tile([128, IH], dt.float32, tag="seTia")
            nc.vector.tensor_copy(seTi_a, seTi_a_ps)
            seTi_b = scr.tile([16, IH], dt.float32, tag="seTib")
            nc.vector.tensor_copy(seTi_b, seTi_b_ps)

            # ---- Co halves -> f16 hi/lo -> R_ALL rows 0/1 ----
            for h in range(2):
                co_ps = ps_all[:, 1024 + 512 * h : 1024 + 512 * h + D_PAIR]
                sl = slice(h * 128, (h + 1) * 128)
                nc.tensor.matmul(co_ps, seT_a[:, sl], w1a, start=True, stop=False)
                nc.tensor.matmul(co_ps, seT_b[:, sl], w1b, start=False, stop=False)
                for c in range(3):
                    nc.tensor.matmul(
                        co_ps,
                        ohP[c][:, sl],
                        posR[:, c * D_PAIR : (c + 1) * D_PAIR],
                        start=False,
                        stop=(c == 2),
                    )
                co_hi = scr.tile(
                    [128, D_PAIR], dt.float16, tag=f"cohi{h}", name=f"cohi{h}"
                )
                nc.vector.tensor_copy(co_hi, co_ps)
                co_lo = scr.tile(
                    [128, D_PAIR], dt.float16, tag=f"colo{h}", name=f"colo{h}"
                )
                nc.vector.tensor_sub(co_lo, co_ps, co_hi)
                dst = slice(h * 128 * D_PAIR, (h * 128 + 128) * D_PAIR)
                nc.sync.dma_start(rall_t[0:1, dst], co_hi)
                nc.sync.dma_start(rall_t[1:2, dst], co_lo)

            # ---- Cp = se_i @ W2 + b_proj + [pe_i | 0] -> cp_rep (x4 f16) ----
            ones_f = scr.tile([1, IH], dt.float32, tag="onesf")
            nc.vector.memset(ones_f, 1.0)
            cp_ps = ps_all[:, 2048 : 2048 + D_PAIR]
            nc.tensor.matmul(cp_ps, seTi_a, w2a, start=True, stop=False)
            nc.tensor.matmul(cp_ps, seTi_b, w2b, start=False, stop=False)
            for c in range(3):
                nc.tensor.matmul(
                    cp_ps,
                    ohPi[c],
                    posL[:, c * D_PAIR : (c + 1) * D_PAIR],
                    start=False,
                    stop=False,
                )
            nc.tensor.matmul(cp_ps, ones_f, bp_t, start=False, stop=True)
            nc.vector.tensor_copy(cp_rep[:, 0:D_PAIR], cp_ps)
            nc.vector.tensor_copy(cp_rep[:, D_PAIR : 2 * D_PAIR], cp_rep[:, 0:D_PAIR])
            nc.vector.tensor_copy(
                cp_rep[:, 2 * D_PAIR : 4 * D_PAIR], cp_rep[:, 0 : 2 * D_PAIR]
            )

            # ---- wsep pattern rows 2..5 of R_ALL ----
            # compute engines must start at partition 0/32/64/96, so build the
            # seed block at partitions 0..3 and DMA it into rows 2..5
            wsep16 = scr.tile([1, D_PAIR], dt.float16, tag="wsep16")
            nc.vector.tensor_copy(wsep16, wsep_f)
            pat4 = scr.tile([4, JG * D_PAIR], dt.float16, tag="pat4")
            nc.vector.memset(pat4, 0.0)
            for jj in range(JG):
                nc.sync.dma_start(
                    pat4[jj : jj + 1, jj * D_PAIR : (jj + 1) * D_PAIR], wsep16
                )
            nc.sync.dma_start(rall_t[2:6, 0 : JG * D_PAIR], pat4)
            # doubling: [4, 1152] -> [4, 73728]; step stays pattern-aligned
            # (multiple of 1152) and under the 64KB SDMA descriptor cap
            x = JG * D_PAIR
            while x < L * D_PAIR:
                step = min(x, L * D_PAIR - x, 28 * JG * D_PAIR)
                nc.sync.dma_start(rall_t[2:6, x : x + step], rall_t[2:6, 0:step])
                x += step

            # ---- sep rows of Gall: row 2+jj, cols g*128+i = ln(|aa_{4g+jj}-aa_i|+1) ----
            for jj in range(JG):
                d_jj = scr.tile([64, IH], dt.float32, tag=f"djj{jj}", name=f"djj{jj}")
                nc.vector.tensor_scalar(
                    d_jj, aaIB_f[0:64, :], aaB4_f[:, jj : jj + 1], None, ALU.subtract
                )
                a_jj = scr.tile([64, IH], dt.float32, tag=f"ajj{jj}", name=f"ajj{jj}")
                nc.scalar.activation(a_jj, d_jj, AF.Abs)
                s_jj = scr.tile([64, IH], dt.float32, tag=f"sjj{jj}", name=f"sjj{jj}")
                nc.scalar.activation(s_jj, a_jj, AF.Ln, bias=1.0)
                s16_jj = scr.tile(
                    [64, IH], dt.float16, tag=f"s16jj{jj}", name=f"s16jj{jj}"
                )
                nc.vector.tensor_copy(s16_jj, s_jj)
                nc.sync.dma_start(gall_t[2 + jj : 3 + jj, :], s16_jj)

        if stage == "setup":
            dbg = ctx.enter_context(tc.tile_pool(name="dbg", bufs=1))
            dbf = dbg.tile([IH, JG * D_PAIR], dt.float16, tag="dbf")
            nc.vector.tensor_copy(dbf, cp_rep)
            nc.sync.dma_start(out_d[:, 0:JG, :], dbf)
            return nc

        # ---- j loop ----
        obp = ctx.enter_context(tc.tile_pool(name="obp", bufs=3))
        cobp = ctx.enter_context(tc.tile_pool(name="cobp", bufs=2))
        ngroups = int(stage[5:]) if stage.startswith("jloop") else NG
        if variant == "dmaonly":
            obs = []
            for k in range(2):
                t = obp.tile([IH, JG * D_PAIR], dt.float16, tag="ob", name="ob")
                nc.vector.memset(t, 0.5)
                obs.append(t)
            for g in range(ngroups * repeat):
                g = g % ngroups
                nc.sync.dma_start(out_d[:, g * JG : (g + 1) * JG, :], obs[g % 2])
            return nc

        for gi in range(ngroups * repeat):
            g = gi % ngroups
            half = gi % 2
            slot0 = 2048 * half
            for jj in range(JG):
                j = g * JG + jj
                ps_j = ps_all[:, slot0 + 512 * jj : slot0 + 512 * jj + D_PAIR]
                nc.tensor.matmul(
                    ps_j,
                    gall_t[0:6, g * IH : (g + 1) * IH],
                    rall_t[0:6, j * D_PAIR : (j + 1) * D_PAIR],
                    start=True,
                    stop=True,
                )
            conv_src = _fstride(
                ps_all[:, slot0 : slot0 + JG * D_PAIR], 512, JG, D_PAIR
            )
            ob = obp.tile([IH, JG * D_PAIR], dt.float16, tag="ob", name="ob")
            mode = sched[g]
            if mode == "D":
                nc.vector.tensor_tensor(ob, conv_src, cp_rep, ALU.add)
            else:
                cob = cobp.tile([IH, JG * D_PAIR], dt.float16, tag="cob", name="cob")
                nc.scalar.copy(cob, conv_src)
                eng = nc.gpsimd if mode == "H" else nc.vector
                eng.tensor_tensor(ob, cob, cp_rep, ALU.add)
            if variant != "nodma":
                nc.sync.dma_start(out_d[:, g * JG : (g + 1) * JG, :], ob)

    return nc


_NC_CACHE = []


def make_in_maps(seq, aa_idx, emb_table, W_proj, b_proj):
    seq = np.asarray(seq, dtype=np.int32)
    aa_idx = np.asarray(aa_idx, dtype=np.int32)
    emb_table = np.ascontiguousarray(np.asarray(emb_table, dtype=np.float32))
    W_proj = np.ascontiguousarray(np.asarray(W_proj, dtype=np.float32))
    b_proj = np.ascontiguousarray(np.asarray(b_proj, dtype=np.float32))
    in_maps = []
    for c in range(N_CORES):
        b, ih = c // 2, c % 2
        in_maps.append(
            {
                "seqb": np.ascontiguousarray(seq[b]),
                "seqi": np.ascontiguousarray(seq[b, ih * IH : (ih + 1) * IH]),
                "aab": np.ascontiguousarray(aa_idx[b]),
                "aai": np.ascontiguousarray(aa_idx[b, ih * IH : (ih + 1) * IH]),
                "emb": emb_table,
                "wp": W_proj,
                "bp": b_proj,
            }
        )
    return in_maps


def gather_out(results) -> np.ndarray:
    out = np.empty((B, L, L, D_PAIR), dtype=np.float32)
    for c in range(N_CORES):
        b, ih = c // 2, c % 2
        out[b, ih * IH : (ih + 1) * IH] = np.asarray(results[c]["out"]).astype(
            np.float32
        )
    return out


def kernel(seq, aa_idx, emb_table, W_proj, b_proj) -> np.ndarray:
    if not _NC_CACHE:
        nc = build()
        nc.finalize()
        _NC_CACHE.append(nc)
    nc = _NC_CACHE[0]
    in_maps = make_in_maps(seq, aa_idx, emb_table, W_proj, b_proj)
    res = run_bass_kernel_spmd(nc, in_maps, core_ids=list(range(N_CORES)))
    return gather_out(res.results)


# revision 33
# speedup vs baseline: 3.6541x; 2.0312x over previous
"""PairEmbedding Bass kernel for 8 TRN2 NeuronCores.

out[b,i,j,:] = Co[b,j,:] + Cp[b,i,:] + sep(b,i,j) * w_sep
  Co[j] = se_j @ W1 + [0 | pe_j]
  Cp[i] = se_i @ W2 + b_proj + [pe_i | 0]
  sep(i,j) = ln(|aa_i - aa_j| + 1)
where se = emb_table[seq], pe = pos_table[aa_idx], W1 = W_proj[0:144],
W2 = W_proj[144:288], w_sep = W_proj[288].

Sharding: core c -> batch b = c//2, row block i in [128*(c%2), 128*(c%2)+128),
all 256 j. Per-core output (128, 256*288) f16 (upcast to f32 on host).

Steady state, per 4096-f32-column cycle (18 cycles cover the 73728-column
flat (j,d) space):
  PE: ~9 matmuls of N<=512 (PSUM bank-aligned pieces, split at 2304-column
      group boundaries). lhsT = Gall group slice (K=9: [1; sep rows for the
      group's 8 j's]); rhs = R_ALL columns ([co f16; 8 wsep pattern rows]).
      Gives Co[j] + sep[:,j]*wsep in f32 PSUM. Pattern rows are zero
      outside their j's columns, so any piece inside a group works with
      the group's lhsT.
  ACT: converts PSUM->f16 for the two 1536-wide units of the cycle.
  DVE: adds cp (phase-shifted periodic replica) to those, and does a fused
      convert+add for the 1024-wide unit straight from PSUM.
  SP: one DMA of the assembled [128, 4096] f16 tile per cycle.
Engine budget per pass: PE ~58us (35us streaming + ~140ns/matmul weight
load), DMA ~53us (f16 output at ~360 GB/s), ACT ~53us, DVE ~54us.
GPSIMD is unused: it cannot read PSUM and its adds measure far slower
than modeled.
"""

import math
from contextlib import ExitStack

import numpy as np

from concourse import bacc, bass, mybir, tile
from concourse.bass_utils import run_bass_kernel_spmd

dt = mybir.dt
AF = mybir.ActivationFunctionType
ALU = mybir.AluOpType

B = 4
L = 256
D_PAIR = 288
D_HALF = 144
MAX_LEN = 260
VOCAB = 21
IH = 128            # i rows per core
JG = 8              # j's per lhsT group (pattern period 8*288 = 2304)
NGRP = L // JG      # 32 lhsT groups
KR = 1 + JG         # lhsT rows: [co; 8 sep rows] (single-f16 Co is plenty
                    # accurate for the 2e-2 rel-err budget)
GRPW = JG * D_PAIR  # 2304 flat columns per group
FW = L * D_PAIR     # 73728 flat columns
CYC = 4096          # PSUM f32 columns per cycle (8 banks)
NCYC = FW // CYC    # 18 cycles per pass
N_CORES = 8

# per-cycle engine schedule for the 3 units: A = ACT convert + DVE f16 add,
# D = DVE fused convert+add from PSUM. GPSIMD can't read PSUM and its adds
# measure far slower than modeled, so it is unused.
SCHED3 = "AAD"


def _pos_enc_table() -> np.ndarray:
    idx = np.arange(0, D_HALF, 2, dtype=np.float32)
    t = (np.float32(math.log(10000.0)) * idx) / np.float32(D_HALF)
    denom = np.exp(t, dtype=np.float32)
    pos = np.arange(MAX_LEN, dtype=np.float32)[:, None]
    pe = np.zeros((MAX_LEN, D_HALF), dtype=np.float32)
    pe[:, 0::2] = np.sin(pos / denom, dtype=np.float32)
    pe[:, 1::2] = np.cos(pos / denom, dtype=np.float32)
    return pe


def _bcast(ap_src, nparts: int):
    return bass.AP(
        tensor=ap_src.tensor, offset=ap_src.offset, ap=[[0, nparts], *ap_src.ap]
    )


def build(stage: str = "full", repeat: int = 1, variant: str = "") -> bass.Bass:
    nc = bacc.Bacc("TRN2", target_bir_lowering=False)

    seqb_d = nc.dram_tensor("seqb", [L], dt.int32, kind="ExternalInput")
    seqi_d = nc.dram_tensor("seqi", [IH], dt.int32, kind="ExternalInput")
    aab_d = nc.dram_tensor("aab", [L], dt.int32, kind="ExternalInput")
    aai_d = nc.dram_tensor("aai", [IH], dt.int32, kind="ExternalInput")
    emb_d = nc.dram_tensor("emb", [VOCAB, D_HALF], dt.float32, kind="ExternalInput")
    wp_d = nc.dram_tensor("wp", [D_PAIR + 1, D_PAIR], dt.float32, kind="ExternalInput")
    bp_d = nc.dram_tensor("bp", [D_PAIR], dt.float32, kind="ExternalInput")
    out_d = nc.dram_tensor("out", [IH, FW], dt.float16, kind="ExternalOutput")

    # pos-table gather sources, pre-arranged on host: chunk c of <=128 pos
    # rows on partitions, channel slice [0:144] (posL, pe_i) or [144:288]
    # (posR, pe_j), zero elsewhere.
    pos_np = _pos_enc_table()
    posL_np = np.zeros((128, 3 * D_PAIR), dtype=np.float32)
    posR_np = np.zeros((128, 3 * D_PAIR), dtype=np.float32)
    for c in range(3):
        rows = 128 if c < 2 else MAX_LEN - 256
        chunk = pos_np[c * 128 : c * 128 + rows, :]
        posL_np[0:rows, c * D_PAIR : c * D_PAIR + D_HALF] = chunk
        posR_np[0:rows, c * D_PAIR + D_HALF : (c + 1) * D_PAIR] = chunk
    posL_d = nc.inline_tensor(posL_np, "posL_c")
    posR_d = nc.inline_tensor(posR_np, "posR_c")
    iota_np = (
        np.arange(128, dtype=np.float32)[:, None]
        + 128.0 * np.arange(3, dtype=np.float32)[None, :]
    ).astype(np.float32)
    iota_d = nc.inline_tensor(iota_np, "iota")
    ones1_d = nc.inline_tensor(np.ones((1, NGRP * IH), dtype=np.float16), "ones1")

    sched = SCHED3
    for v in variant.split("+"):
        if v.startswith("sched"):
            sched = v[5:]
    assert len(sched) == 3
    # conv/add units per cycle: the D (DVE-fused) unit is 1024 wide, A units
    # 1536; unit order follows the sched string
    units = []
    lo = 0
    for ch in sched:
        w = 1024 if ch == "D" else 1536
        units.append((lo, lo + w))
        lo += w
    assert lo == CYC, sched

    with tile.TileContext(nc) as tc, ExitStack() as ctx:
        persist = ctx.enter_context(tc.tile_pool(name="persist", bufs=1))
        psp = ctx.enter_context(tc.tile_pool(name="psp", bufs=1, space="PSUM"))

        # R_ALL rows: 0=co (f16), 1..8 = wsep at j%8==jj
        rall_t = persist.tile([KR, FW], dt.float16, tag="rall")
        # Gall rows: 0=ones, 1..8 = sep[i, 8g+jj]; free = g*128 + i
        gall_t = persist.tile([KR, NGRP * IH], dt.float16, tag="gall")
        # periodic cp replica: cp_rep[:, c] = cp[:, c % 288], c < 288+1536
        CPW = D_PAIR + 1536
        cp_rep = persist.tile([IH, CPW], dt.float16, tag="cprep")
        ps_all = psp.tile([128, CYC], dt.float32, tag="psall")

        with ExitStack() as pre:
            scr = pre.enter_context(tc.tile_pool(name="scr", bufs=1))

            # ---- input loads ----
            iota_t = scr.tile([128, 3], dt.float32, tag="iota")
            nc.sync.dma_start(iota_t, iota_d[:, :])

            emb_t = scr.tile([VOCAB, D_HALF], dt.float32, tag="emb")
            nc.sync.dma_start(emb_t, emb_d[:, :])

            w1a = scr.tile([128, D_PAIR], dt.float32, tag="w1a")
            nc.sync.dma_start(w1a, wp_d[0:128, :])
            w1b = scr.tile([16, D_PAIR], dt.float32, tag="w1b")
            nc.sync.dma_start(w1b, wp_d[128:144, :])
            w2a = scr.tile([128, D_PAIR], dt.float32, tag="w2a")
            nc.sync.dma_start(w2a, wp_d[144:272, :])
            w2b = scr.tile([16, D_PAIR], dt.float32, tag="w2b")
            nc.sync.dma_start(w2b, wp_d[272:288, :])
            wsep_f = scr.tile([1, D_PAIR], dt.float32, tag="wsepf")
            nc.sync.dma_start(wsep_f, wp_d[288:289, :])

            bp_t = scr.tile([1, D_PAIR], dt.float32, tag="bp")
            nc.sync.dma_start(bp_t, bp_d[:])

            seqB_i = scr.tile([VOCAB, L], dt.int32, tag="seqBi")
            nc.sync.dma_start(seqB_i, _bcast(seqb_d[:], VOCAB))
            seqI_i = scr.tile([VOCAB, IH], dt.int32, tag="seqIi")
            nc.sync.dma_start(seqI_i, _bcast(seqi_d[:], VOCAB))
            aaIB_i = scr.tile([128, IH], dt.int32, tag="aaIBi")
            nc.sync.dma_start(aaIB_i, _bcast(aai_d[:], 128))
            aaB_i = scr.tile([128, L], dt.int32, tag="aaBi")
            nc.sync.dma_start(aaB_i, _bcast(aab_d[:], 128))
            # aaB8[jj, g] = aa[8g+jj]
            aaB8_i = scr.tile([JG, NGRP], dt.int32, tag="aaB8i")
            nc.sync.dma_start(
                aaB8_i,
                bass.AP(tensor=aab_d[:].tensor, offset=0, ap=[[1, JG], [JG, NGRP]]),
            )

            posL = scr.tile([128, 3 * D_PAIR], dt.float32, tag="posL")
            nc.sync.dma_start(posL, posL_d[:, :])
            posR = scr.tile([128, 3 * D_PAIR], dt.float32, tag="posR")
            nc.sync.dma_start(posR, posR_d[:, :])

            nc.sync.dma_start(gall_t[0:1, :], ones1_d[:, :])

            # ---- int -> f32 casts ----
            seqB_f = scr.tile([VOCAB, L], dt.float32, tag="seqBf")
            nc.vector.tensor_copy(seqB_f, seqB_i)
            seqI_f = scr.tile([VOCAB, IH], dt.float32, tag="seqIf")
            nc.vector.tensor_copy(seqI_f, seqI_i)
            aaIB_f = scr.tile([128, IH], dt.float32, tag="aaIBf")
            nc.vector.tensor_copy(aaIB_f, aaIB_i)
            aaB_f = scr.tile([128, L], dt.float32, tag="aaBf")
            nc.vector.tensor_copy(aaB_f, aaB_i)
            aaB8_f = scr.tile([JG, NGRP], dt.float32, tag="aaB8f")
            nc.vector.tensor_copy(aaB8_f, aaB8_i)

            # ---- one-hots ----
            ohSeq = scr.tile([VOCAB, L], dt.float32, tag="ohSeq")
            nc.vector.tensor_scalar(
                ohSeq, seqB_f, iota_t[0:VOCAB, 0:1], None, ALU.is_equal
            )
            ohSeqI = scr.tile([VOCAB, IH], dt.float32, tag="ohSeqI")
            nc.vector.tensor_scalar(
                ohSeqI, seqI_f, iota_t[0:VOCAB, 0:1], None, ALU.is_equal
            )
            ohP = []
            ohPi = []
            for c in range(3):
                t = scr.tile([128, L], dt.float32, tag=f"ohP{c}", name=f"ohP{c}")
                nc.vector.tensor_scalar(t, aaB_f, iota_t[:, c : c + 1], None, ALU.is_equal)
                ohP.append(t)
                ti = scr.tile([128, IH], dt.float32, tag=f"ohPi{c}", name=f"ohPi{c}")
                nc.vector.tensor_scalar(
                    ti, aaIB_f, iota_t[:, c : c + 1], None, ALU.is_equal
                )
                ohPi.append(ti)

            # ---- seT = emb^T gathered by seq: (144, L) split 128+16 rows ----
            seT_a_ps = ps_all[:, 0:L]
            nc.tensor.matmul(seT_a_ps, emb_t[:, 0:128], ohSeq, start=True, stop=True)
            seT_b_ps = ps_all[0:16, 256 : 256 + L]
            nc.tensor.matmul(
                seT_b_ps, emb_t[:, 128:D_HALF], ohSeq, start=True, stop=True
            )
            seT_a = scr.tile([128, L], dt.float32, tag="seTa")
            nc.vector.tensor_copy(seT_a, seT_a_ps)
            seT_b = scr.tile([16, L], dt.float32, tag="seTb")
            nc.vector.tensor_copy(seT_b, seT_b_ps)

            seTi_a_ps = ps_all[:, 512 : 512 + IH]
            nc.tensor.matmul(seTi_a_ps, emb_t[:, 0:128], ohSeqI, start=True, stop=True)
            seTi_b_ps = ps_all[0:16, 640 : 640 + IH]
            nc.tensor.matmul(
                seTi_b_ps, emb_t[:, 128:D_HALF], ohSeqI, start=True, stop=True
            )
            seTi_a = scr.tile([128, IH], dt.float32, tag="seTia")
            nc.vector.tensor_copy(seTi_a, seTi_a_ps)
            seTi_b = scr.tile([16, IH], dt.float32, tag="seTib")
            nc.vector.tensor_copy(seTi_b, seTi_b_ps)

            # ---- Co halves -> f16 hi/lo -> R_ALL rows 0/1 ----
            for h in range(2):
                co_ps = ps_all[:, 1024 + 512 * h : 1024 + 512 * h + D_PAIR]
                sl = slice(h * 128, (h + 1) * 128)
                nc.tensor.matmul(co_ps, seT_a[:, sl], w1a, start=True, stop=False)
                nc.tensor.matmul(co_ps, seT_b[:, sl], w1b, start=False, stop=False)
                for c in range(3):
                    nc.tensor.matmul(
                        co_ps,
                        ohP[c][:, sl],
                        posR[:, c * D_PAIR : (c + 1) * D_PAIR],
                        start=False,
                        stop=(c == 2),
                    )
                co_hi = scr.tile(
                    [128, D_PAIR], dt.float16, tag=f"cohi{h}", name=f"cohi{h}"
                )
                nc.vector.tensor_copy(co_hi, co_ps)
                dst = slice(h * 128 * D_PAIR, (h * 128 + 128) * D_PAIR)
                nc.sync.dma_start(rall_t[0:1, dst], co_hi)

            # ---- Cp -> periodic f16 replica cp_rep ----
            ones_f = scr.tile([1, IH], dt.float32, tag="onesf")
            nc.vector.memset(ones_f, 1.0)
            cp_ps = ps_all[:, 2048 : 2048 + D_PAIR]
            nc.tensor.matmul(cp_ps, seTi_a, w2a, start=True, stop=False)
            nc.tensor.matmul(cp_ps, seTi_b, w2b, start=False, stop=False)
            for c in range(3):
                nc.tensor.matmul(
                    cp_ps,
                    ohPi[c],
                    posL[:, c * D_PAIR : (c + 1) * D_PAIR],
                    start=False,
                    stop=False,
                )
            nc.tensor.matmul(cp_ps, ones_f, bp_t, start=False, stop=True)
            nc.vector.tensor_copy(cp_rep[:, 0:D_PAIR], cp_ps)
            x = D_PAIR
            while x < CPW:
                step = min(x, CPW - x)
                nc.vector.tensor_copy(cp_rep[:, x : x + step], cp_rep[:, 0:step])
                x += step

            # ---- wsep pattern rows 1..8 of R_ALL ----
            # zero-fill + wsep seeds via DMA (compute engines can't start at
            # partition 1), then period-doubling DMAs
            wsep16 = scr.tile([1, D_PAIR], dt.float16, tag="wsep16")
            nc.vector.tensor_copy(wsep16, wsep_f)
            zt = scr.tile([JG, 1152], dt.float16, tag="zt")
            nc.vector.memset(zt, 0.0)
            for q in range(GRPW // 1152):
                nc.sync.dma_start(rall_t[1 : 1 + JG, q * 1152 : (q + 1) * 1152], zt)
            for jj in range(JG):
                nc.sync.dma_start(
                    rall_t[1 + jj : 2 + jj, jj * D_PAIR : (jj + 1) * D_PAIR], wsep16
                )
            # doubling; steps stay multiples of the 2304 period and under the
            # 64KB SDMA descriptor cap (<= 32256 f16 = 64512B)
            x = GRPW
            while x < FW:
                step = min(x, FW - x, 14 * GRPW)
                nc.sync.dma_start(
                    rall_t[1 : 1 + JG, x : x + step], rall_t[1 : 1 + JG, 0:step]
                )
                x += step

            # ---- sep rows of Gall: row 1+jj, col g*128+i = ln(|aa_{8g+jj}-aa_i|+1) ----
            s16_all = scr.tile([JG, NGRP * IH], dt.float16, tag="s16all")
            HG = NGRP // 2
            for hh in range(2):
                d_half = scr.tile([JG, HG * IH], dt.float32, tag="dhalf", name="dhalf")
                for gg in range(HG):
                    g = hh * HG + gg
                    nc.vector.tensor_scalar(
                        d_half[:, gg * IH : (gg + 1) * IH],
                        aaIB_f[0:JG, :],
                        aaB8_f[:, g : g + 1],
                        None,
                        ALU.subtract,
                    )
                nc.scalar.activation(d_half, d_half, AF.Abs)
                nc.scalar.activation(d_half, d_half, AF.Ln, bias=1.0)
                nc.vector.tensor_copy(
                    s16_all[:, hh * HG * IH : (hh + 1) * HG * IH], d_half
                )
            nc.sync.dma_start(gall_t[1 : 1 + JG, :], s16_all)

        if stage == "setup":
            dbg = ctx.enter_context(tc.tile_pool(name="dbg", bufs=1))
            dbf = dbg.tile([IH, CPW], dt.float16, tag="dbf")
            nc.vector.tensor_copy(dbf, cp_rep)
            nc.sync.dma_start(out_d[:, 0:CPW], dbf)
            return nc

        # ---- steady loop ----
        obp = ctx.enter_context(tc.tile_pool(name="obp", bufs=4))
        cobp = ctx.enter_context(tc.tile_pool(name="cobp", bufs=5))
        ncyc = int(stage[5:]) if stage.startswith("jloop") else NCYC
        noconv = "noconv" in variant
        nodma = "nodma" in variant
        if variant == "dmaonly":
            obs = []
            for k in range(2):
                t = obp.tile([IH, CYC], dt.float16, tag="ob", name="ob")
                nc.vector.memset(t, 0.5)
                obs.append(t)
            for k in range(ncyc * repeat):
                k = k % ncyc
                nc.sync.dma_start(out_d[:, k * CYC : (k + 1) * CYC], obs[k % 2])
            return nc

        obs = []
        if noconv:
            for k in range(2):
                t = obp.tile([IH, CYC], dt.float16, tag="ob", name="ob")
                nc.vector.memset(t, 0.5)
                obs.append(t)
        # matmul pieces per cycle: 512-bank chunks, split where a 2304-column
        # group boundary falls inside a chunk (the lhsT sep rows only match
        # one group's j's)
        cycle_pieces = []
        for k in range(NCYC):
            pieces = []
            for c in range(8):
                lo = k * CYC + 512 * c
                hi = lo + 512
                b = (lo // GRPW + 1) * GRPW
                if b < hi:
                    pieces.append((lo, b))
                    lo = b
                pieces.append((lo, hi))
            cycle_pieces.append(pieces)

        for ki in range(ncyc * repeat):
            k = ki % ncyc
            base = k * CYC  # flat column base of this cycle
            for lo, hi in cycle_pieces[k]:
                g = lo // GRPW
                nc.tensor.matmul(
                    ps_all[:, lo - base : hi - base],
                    gall_t[:, g * IH : (g + 1) * IH],
                    rall_t[:, lo:hi],
                    start=True,
                    stop=True,
                )
            if noconv:
                if not nodma:
                    nc.sync.dma_start(
                        out_d[:, base : base + CYC], obs[ki % 2]
                    )
                continue
            ob = obp.tile([IH, CYC], dt.float16, tag="ob", name="ob")
            for u, (lo, hi) in enumerate(units):
                w = hi - lo
                phase = (base + lo) % D_PAIR
                cps = cp_rep[:, phase : phase + w]
                if sched[u] == "D":
                    nc.vector.tensor_tensor(
                        ob[:, lo:hi], ps_all[:, lo:hi], cps, ALU.add
                    )
                else:
                    cob = cobp.tile([IH, 1536], dt.float16, tag="cob", name="cob")
                    nc.scalar.copy(cob[:, 0:w], ps_all[:, lo:hi])
                    nc.vector.tensor_tensor(ob[:, lo:hi], cob[:, 0:w], cps, ALU.add)
            if not nodma:
                nc.sync.dma_start(out_d[:, base : base + CYC], ob)

    return nc


_NC_CACHE = []


def make_in_maps(seq, aa_idx, emb_table, W_proj, b_proj):
    seq = np.asarray(seq, dtype=np.int32)
    aa_idx = np.asarray(aa_idx, dtype=np.int32)
    emb_table = np.ascontiguousarray(np.asarray(emb_table, dtype=np.float32))
    W_proj = np.ascontiguousarray(np.asarray(W_proj, dtype=np.float32))
    b_proj = np.ascontiguousarray(np.asarray(b_proj, dtype=np.float32))
    in_maps = []
    for c in range(N_CORES):
        b, ih = c // 2, c % 2
        in_maps.append(
            {
                "seqb": np.ascontiguousarray(seq[b]),
                "seqi": np.ascontiguousarray(seq[b, ih * IH : (ih + 1) * IH]),
                "aab": np.ascontiguousarray(aa_idx[b]),
                "aai": np.ascontiguousarray(aa_idx[b, ih * IH : (ih + 1) * IH]),
                "emb": emb_table,
                "wp": W_proj,
                "bp": b_proj,
            }
        )
    return in_maps


def gather_out(results) -> np.ndarray:
    out = np.empty((B, L, L, D_PAIR), dtype=np.float32)
    for c in range(N_CORES):
        b, ih = c // 2, c % 2
        out[b, ih * IH : (ih + 1) * IH] = (
            np.asarray(results[c]["out"]).astype(np.float32).reshape(IH, L, D_PAIR)
        )
    return out


def kernel(seq, aa_idx, emb_table, W_proj, b_proj) -> np.ndarray:
    if not _NC_CACHE:
        nc = build()
        nc.finalize()
        _NC_CACHE.append(nc)
    nc = _NC_CACHE[0]
    in_maps = make_in_maps(seq, aa_idx, emb_table, W_proj, b_proj)
    res = run_bass_kernel_spmd(nc, in_maps, core_ids=list(range(N_CORES)))
    return gather_out(res.results)
